# revision 67
# baseline (speedup 1.0000x reference)
"""Trainium2 Bass kernel for nn_EnergyLoss: batched 16x16 complex Hermitian
ground-state projector via shifted matrix-squaring power iteration.

Math (derived from the reference):
  H[n] = 0.5*G - 0.5*sum_d X[n,d]*S_d + (0.5*q_n + EPS)*I,
     G = sum_d A_d A_d^H,  S_d = A_d + A_d^H,  q_n = sum_d X[n,d]^2
  B0 = PF*(I - H/||H||_F)   (PSD shift, prefolded by PF=1/3 so ||B0||_F ~ 1)
  B <- B^2, renormalized by 1/||B||_F^2 on steps {2,5,8}   (12 steps total)
  B converges to c*P (ground-state projector); loss terms from P via rowsums.

Implementation notes:
  - complex 16x16 embedded as real 32x32 M(B) = [[Br,-Bi],[Bi,Br]]; 4 samples
    stacked per 128 partitions; per-sample squaring = one 32x32 PE-tile matmul
    with 16-col moving operand ([Br;Bi] half of M).
  - state per step: wb [128, 32*NQ] f16 holds full M; left 16 cols/quad are
    the t-form [Br;Bi] (cast directly from PSUM), right 16 cols [-Bi;Br] are
    stream-shuffled from u = t*signp.
  - ||H||_F^2 computed as y^T G34 y (G34 precomputed host-side, 34-dim
    y=(x,1,q)); fro itself feeds back as a 35th contraction row so the H-build
    matmul emits PF*(fro*I - H) directly (no separate diagonal add).
  - 4-slab pipelining keeps PE (the bottleneck) continuously fed; elementwise
    work is split across Act/DVE/Pool so each stays under the PE step time.
"""

import numpy as np

N, D, DIM = 4096, 32, 16
NCORES = 8
NS = N // NCORES          # 512 samples per core
NQ = NS // 4              # 128 quads (4 samples per 128 partitions)
EPS = 1e-5
LAM = 0.1
PF = 1.0 / 3.0            # prefold of B0
KSTEPS = 11
NORM_STEPS = (2, 5, 8)
NSLAB = 4
QS = NQ // NSLAB          # 32 quads per slab
GSH = 8                   # G34 scaled by 2^-GSH to keep f16 products in range

_prog_cache = {}

# ---- cinA packed layout (bytes per partition) -----------------------------
A_XTH = 0                 # f16 [35, 512]   1024B  (row 34 device-written fro)
A_WH = 1024               # f16 [35, 512]   1024B
A_G34 = 2048              # f16 [34, 64]    128B   (34 used, padded)
A_ONES = 2176             # f16 [34, 128]   256B
A_SIGNP = 2432            # f32 [128, 1]    4B (pad 16)
A_MASKB = 2448            # f32 [128, 128]  512B
CA = 2960
# ---- cinB ------------------------------------------------------------------
B_WPOS = 0                # f32 [128, 128]  512B
B_WEA2 = 512              # f32 [128, 128]  512B
B_XBLK = 1024             # f32 [128, 128]  512B
CB = 1536


def _build_host_tensors(A_real, A_imag, X):
    A = (A_real + 1j * A_imag).astype(np.complex64)
    Sc = A + np.conj(np.transpose(A, (0, 2, 1)))        # [D,16,16] Hermitian
    Sr, Si = Sc.real.astype(np.float64), Sc.imag.astype(np.float64)
    G = np.einsum('dij,dkj->ik', A, A.conj())
    Gr, Gi = G.real.astype(np.float64), G.imag.astype(np.float64)
    cA = A.sum(axis=1)                                   # [D,16] colsum over i
    cA2 = (A @ A).sum(axis=1)

    # WH[k, 32j+m]: contraction k: 0 = fro (device-written), 1..32 = d,
    # 33 = const, 34 = q.  Emits pm = PF*(fro*I - H) in t-layout.
    WH = np.zeros((35, 512), np.float64)
    for j in range(DIM):
        c = 32 * j
        WH[0, c+j] = 1.0                                 # fro * I
        WH[1:1+D, c:c+16] = 0.5 * Sr[:, :, j]            # -(-0.5 Sr) = +0.5
        WH[1:1+D, c+16:c+32] = 0.5 * Si[:, :, j]
        WH[33, c:c+16] = -0.5 * Gr[:, j]
        WH[33, c+j] -= EPS
        WH[33, c+16:c+32] = -0.5 * Gi[:, j]
        WH[34, c+j] = -0.5
    WH *= PF

    # G35 for fro^2 = y^T G y (y = rows 1..34 of xth: (x, 1, q)); row/col 0
    # (the fro row) is zero so the runtime fro value never contributes.
    Wraw = WH[1:35].reshape(34, 16, 32) / PF
    G34 = np.einsum('kjm,ljm->kl', Wraw, Wraw) * (2.0 ** -GSH)
    G35 = np.zeros((35, 35))
    G35[1:35, 1:35] = G34

    MASKB = np.zeros((128, 128), np.float32)
    for b in range(4):
        MASKB[32*b:32*b+32, 32*b:32*b+32] = 1.0
    SIGNP = np.ones((128, 1), np.float32)
    for s in range(4):
        SIGNP[32*s+16:32*s+32, 0] = -1.0
    # finish functionals: rs is rowsums of t-form [Pr; +Pi]
    #   pos_raw[32s+d, q] = sum_i cAr[d,i]*rr[i] - cAi[d,i]*ri[i]
    WPOS = np.zeros((128, 128), np.float32)
    WEA2 = np.zeros((128, 128), np.float32)
    for s in range(4):
        b = 32 * s
        WPOS[b:b+16, b:b+32] = cA.real.T
        WPOS[b+16:b+32, b:b+32] = -cA.imag.T
        WEA2[b:b+16, b:b+32] = cA2.real.T
        WEA2[b+16:b+32, b:b+32] = -cA2.imag.T
    ONES34 = np.ones((35, 128), np.float32)

    def put(buf, off, arr, dt):
        a = np.ascontiguousarray(arr.astype(dt))
        b = a.view(np.uint8).reshape(a.shape[0], -1)
        buf[:a.shape[0], off:off+b.shape[1]] = b

    per_core = []
    for c in range(NCORES):
        Xc = np.asarray(X[c*NS:(c+1)*NS], np.float64)    # [512, 32]
        q = (Xc ** 2).sum(1)
        XTH = np.zeros((35, 512), np.float64)
        XBLK = np.zeros((128, 128), np.float32)
        for s in range(4):
            idx = np.arange(NQ) * 4 + s                  # sample (q, s)
            XTH[1:1+D, 128*s:128*s+128] = Xc[idx].T
            XTH[33, 128*s:128*s+128] = 1.0
            XTH[34, 128*s:128*s+128] = q[idx]
            XBLK[32*s:32*s+32, :] = Xc[idx].T.astype(np.float32)
        bufA = np.zeros((128, CA), np.uint8)
        put(bufA, A_XTH, XTH, np.float16)
        put(bufA, A_WH, WH, np.float16)
        g = np.zeros((35, 64), np.float64)
        g[:, :35] = G35
        put(bufA, A_G34, g, np.float16)
        put(bufA, A_ONES, ONES34, np.float16)
        put(bufA, A_SIGNP, SIGNP, np.float32)
        put(bufA, A_MASKB, MASKB, np.float32)
        bufB = np.zeros((128, CB), np.uint8)
        put(bufB, B_WPOS, WPOS, np.float32)
        put(bufB, B_WEA2, WEA2, np.float32)
        put(bufB, B_XBLK, XBLK, np.float32)
        per_core.append({"cina": bufA, "cinb": bufB})
    return per_core


def build_program(ksteps=KSTEPS, norm_steps=NORM_STEPS, debug=False):
    import concourse.bass as bass
    import concourse.bass_isa as bass_isa
    import concourse.bacc as bacc
    import concourse.mybir as mybir
    import concourse.tile as tile
    from contextlib import ExitStack

    f16, f32 = mybir.dt.float16, mybir.dt.float32
    u8, u32 = mybir.dt.uint8, mybir.dt.uint32
    Alu = mybir.AluOpType
    Act = mybir.ActivationFunctionType
    X_AX = mybir.AxisListType.X
    HSWAP = list(range(16, 32)) + list(range(0, 16))
    norm_set = set(norm_steps)

    nc = bacc.Bacc()
    d_cina = nc.dram_tensor("cina", [128, CA], u8, kind="ExternalInput")
    d_cinb = nc.dram_tensor("cinb", [128, CB], u8, kind="ExternalInput")
    d_out = nc.dram_tensor("out", [128, NQ], f32, kind="ExternalOutput")
    if debug:
        d_dbg_fro = nc.dram_tensor("dbg_fro", [128, 512], f32,
                                   kind="ExternalOutput")
        d_dbg_invf = nc.dram_tensor("dbg_invf", [128, 128], f32,
                                    kind="ExternalOutput")
        d_dbg_wb0 = nc.dram_tensor("dbg_wb0", [128, 4096], f16,
                                   kind="ExternalOutput")
        d_dbg_wb1 = nc.dram_tensor("dbg_wb1", [128, 4096], f16,
                                   kind="ExternalOutput")
        d_dbg_rs = nc.dram_tensor("dbg_rs", [128, 128], f32,
                                  kind="ExternalOutput")
        d_dbg_prt = nc.dram_tensor("dbg_prt", [128, 128], f32,
                                   kind="ExternalOutput")
        d_dbg_wb2 = nc.dram_tensor("dbg_wb2", [128, 4096], f16,
                                   kind="ExternalOutput")
        d_dbg_wb3 = nc.dram_tensor("dbg_wb3", [128, 4096], f16,
                                   kind="ExternalOutput")
        d_dbg_scl = nc.dram_tensor("dbg_scl", [128, 128], f32,
                                   kind="ExternalOutput")
        d_dbg_wbs = {
            k: nc.dram_tensor(f"dbg_wbs{k}", [128, 4096], f16,
                              kind="ExternalOutput")
            for k in range(3, KSTEPS)
        }

    with tile.TileContext(nc) as tc, ExitStack() as ctx:
        cpool = ctx.enter_context(tc.tile_pool(name="consts", bufs=1))
        spool = ctx.enter_context(tc.tile_pool(name="state", bufs=3))
        wpool = ctx.enter_context(tc.tile_pool(name="work", bufs=3))
        upool = ctx.enter_context(tc.tile_pool(name="uslab", bufs=6))
        qpool = ctx.enter_context(tc.tile_pool(name="small", bufs=3))
        pmpool = ctx.enter_context(tc.tile_pool(name="psum_pm", bufs=4,
                                                space="PSUM"))
        p1ctx = ExitStack()
        p1pool = p1ctx.enter_context(tc.tile_pool(name="psum_p1", bufs=1,
                                                  space="PSUM"))

        cina = cpool.tile([128, CA], u8, tag="cina")
        cinb = cpool.tile([128, CB], u8, tag="cinb")
        nc.sync.dma_start(cina[:, :], d_cina[:, :])
        nc.sync.dma_start(cinb[:, :], d_cinb[:, :])

        xth = cina[:, A_XTH:A_XTH+1024].bitcast(f16)[0:35, :]
        wh = cina[:, A_WH:A_WH+1024].bitcast(f16)[0:35, :]
        g34 = cina[:, A_G34:A_G34+128].bitcast(f16)[0:35, :]
        ones34 = cina[:, A_ONES:A_ONES+256].bitcast(f16)[0:35, :]
        signp = cina[:, A_SIGNP:A_SIGNP+4].bitcast(f32)
        maskb = cina[:, A_MASKB:A_MASKB+512].bitcast(f32)
        wpos = cinb[:, B_WPOS:B_WPOS+512].bitcast(f32)
        wea2 = cinb[:, B_WEA2:B_WEA2+512].bitcast(f32)
        xblk = cinb[:, B_XBLK:B_XBLK+512].bitcast(f32)

        # ---------------- warmup: keep PE busy from t=0 --------------------
        wz = wpool.tile([128, 384], f16, tag="wz")
        nc.vector.memset(wz[:, :], 0.0)
        # preload activation function tables off the critical path
        wact = wpool.tile([128, 16], f32, tag="wact")
        nc.scalar.activation(wact[:, :], wz[:, 0:16], Act.Copy)
        nc.scalar.activation(wact[:, :], wz[:, 0:16], Act.Square)
        nc.scalar.activation(wact[:, :], wz[:, 0:16], Act.Sqrt)
        pwarm = p1pool.tile([128, 512], f32, tag="warm")
        for _ in range(8):
            nc.tensor.matmul(pwarm[:, 0:256], wz[0:32, 0:128],
                             wz[0:32, 128:384], start=True, stop=True)

        # ---------------- fro chain: fro^2 = y^T G34 y ---------------------
        gy = p1pool.tile([35, 512], f32, tag="gy")
        nc.tensor.matmul(gy[:, :], g34[:, 0:35], xth[:, :],
                         start=True, stop=True)
        for _ in range(2):
            nc.tensor.matmul(pwarm[:, 0:256], wz[0:32, 0:128],
                             wz[0:32, 128:384], start=True, stop=True)
        prod = wpool.tile([35, 512], f16, tag="prod")
        nc.vector.tensor_tensor(prod[:, :], gy[:, :], xth[:, :],
                                op=Alu.mult)
        fro2b = p1pool.tile([128, 512], f32, tag="fro2b")
        nc.tensor.matmul(fro2b[:, :], ones34[:, :], prod[:, :],
                         start=True, stop=True)
        for _ in range(8):
            nc.tensor.matmul(pwarm[:, 0:256], wz[0:32, 0:128],
                             wz[0:32, 128:384], start=True, stop=True)
        # xth row 0 <- fro = sqrt(fro2b * 2^GSH)
        nc.scalar.activation(xth[0:1, :], fro2b[0:1, :], Act.Sqrt,
                             scale=float(2.0 ** GSH))
        # invf[p, q] = 1/fro of sample (q, s(p))
        invsq = wpool.tile([128, 128], f32, tag="invsq")
        for s in range(4):
            nc.vector.reciprocal(invsq[32*s:32*s+32, :],
                                 fro2b[32*s:32*s+32, 128*s:128*s+128])
        invf = wpool.tile([128, 128], f32, tag="invf")
        nc.scalar.activation(invf[:, :], invsq[:, :], Act.Sqrt,
                             scale=float(2.0 ** -GSH))

        # ---------------- helpers -----------------------------------------
        def wb_left(wb_t, sl):
            return wb_t[:, :].rearrange("p (q c) -> p q c", c=32)[
                :, sl*QS:(sl+1)*QS, 0:16]

        def wb_right_u32(wb_t, sl):
            return wb_t[:, :].bitcast(u32).rearrange("p (q w) -> p q w", w=16)[
                :, sl*QS:(sl+1)*QS, 8:16]

        def emit_trio(wbn, pm_t, sl, scl=None, cast_engine="act",
                      last=False):
            """pm [128, 16*QS] -> wbn left (t-form), u, wbn right."""
            dst = wb_left(wbn, sl)
            src = pm_t[:, :].rearrange("p (q j) -> p q j", j=16)
            if scl is not None:
                nc.vector.tensor_tensor(
                    dst, src,
                    scl.unsqueeze(-1).broadcast_to([128, QS, 16]),
                    op=Alu.mult)
            elif cast_engine == "act":
                nc.scalar.activation(dst, src, Act.Copy)
            else:
                nc.vector.tensor_copy(dst, src)
            if last:
                return
            us = upool.tile([128, 16*QS], f16, tag=f"u{sl % 2}")
            nc.vector.tensor_scalar_mul(
                us[:, :].rearrange("p (q j) -> p q j", j=16),
                wb_left(wbn, sl), signp[:, :])
            nc.vector.stream_shuffle(
                wb_right_u32(wbn, sl),
                us[:, :].bitcast(u32).rearrange("p (q w) -> p q w", w=8),
                mask=HSWAP)

        def emit_prep_sq(pm_t, sq_t, wbn_t=None, sl=0):
            """squares for fro^2, straight from the squaring PSUM (Act)."""
            nc.scalar.activation(
                sq_t[:, :].rearrange("p (q j) -> p q j", j=16),
                pm_t[:, :].rearrange("p (q j) -> p q j", j=16), Act.Square)

        def emit_prep_red(sq_t, pr_t):
            """j-reduce of the squares -> per-partition partials."""
            nc.vector.tensor_reduce(
                pr_t[:, :], sq_t[:, :].rearrange("p (q j) -> p q j", j=16),
                axis=X_AX, op=Alu.add)

        def emit_prep_trp(pr_t, sl, trp_t, inv_t):
            """per-block partition sums (PE) + reciprocal; emitted where
            pr is already complete so the PE stream never blocks."""
            nc.tensor.matmul(trp_t[:, sl*QS:(sl+1)*QS], maskb[:, :],
                             pr_t[:, :], start=True, stop=True)
            nc.vector.reciprocal(inv_t[:, sl*QS:(sl+1)*QS],
                                 trp_t[:, sl*QS:(sl+1)*QS])

        # ---------------- phase 1: H build -> B0 ---------------------------
        wb = spool.tile([128, 32*NQ], f16, tag="wb")
        for sl in range(NSLAB):
            ph = pmpool.tile([128, 16*QS], f32, tag="pm")
            for j in range(DIM):
                for s in range(4):
                    nc.tensor.matmul(
                        ph[32*s:32*s+32, 32*j:32*j+32],
                        wh[:, 32*j:32*j+32],
                        xth[:, 128*s+QS*sl:128*s+QS*sl+QS],
                        start=True, stop=True, tile_position=(0, 32*s))
            # cast1 with per-quad 1/fro (DVE), u on Act, shuffle DVE
            dst = wb_left(wb, sl)
            nc.vector.tensor_tensor(
                dst, ph[:, :].rearrange("p (j q) -> p q j", j=16),
                invf[:, QS*sl:QS*(sl+1)].unsqueeze(-1)
                    .broadcast_to([128, QS, 16]),
                op=Alu.mult)
            us = upool.tile([128, 16*QS], f16, tag=f"u{sl % 2}")
            nc.scalar.activation(us[:, :], wb_left(wb, sl), Act.Copy,
                                 scale=signp[:, :])
            nc.vector.stream_shuffle(
                wb_right_u32(wb, sl),
                us[:, :].bitcast(u32).rearrange("p (q w) -> p q w", w=8),
                mask=HSWAP)

        if debug:
            frocp = wpool.tile([128, 512], f32, tag="frocp")
            nc.vector.tensor_copy(frocp[:, :], fro2b[:, :])
            nc.sync.dma_start(d_dbg_fro[:, :], frocp[:, :])
            nc.sync.dma_start(d_dbg_invf[:, :], invf[:, :])
            nc.sync.dma_start(d_dbg_wb0[:, :], wb[:, :])

        # ---------------- iteration ----------------------------------------
        p1ctx.close()
        smpool = ctx.enter_context(tc.tile_pool(name="psum_sm", bufs=2,
                                                space="PSUM"))
        rs = wpool.tile([128, NQ], f32, tag="rs")
        pos = smpool.tile([128, NQ], f32, tag="fin")
        ea2 = smpool.tile([128, NQ], f32, tag="fin")
        r = wpool.tile([128, NQ], f32, tag="r")
        posn = wpool.tile([128, NQ], f32, tag="posn")
        ea2n = wpool.tile([128, NQ], f32, tag="ea2n")
        terr = wpool.tile([128, NQ], f32, tag="terr")
        t2 = wpool.tile([128, NQ], f32, tag="t2")
        p2 = wpool.tile([128, NQ], f32, tag="p2")
        vv = wpool.tile([128, NQ], f32, tag="vv")
        fin_done = set()

        def emit_finish_rowsum(wb_t, sl):
            nc.vector.tensor_reduce(
                rs[:, sl*QS:(sl+1)*QS], wb_left(wb_t, sl), axis=X_AX,
                op=Alu.add)

        def emit_finish_slab(sl):
            if sl in fin_done:
                return
            fin_done.add(sl)
            c = slice(sl*QS, (sl+1)*QS)
            nc.tensor.matmul(pos[:, c], wpos[:, :], rs[:, c], start=True,
                             stop=True)
            nc.tensor.matmul(ea2[:, c], wea2[:, :], rs[:, c], start=True,
                             stop=True)
            nc.vector.tensor_tensor(posn[:, c], pos[:, c], fin_invt[:, c],
                                    op=Alu.mult)
            nc.vector.tensor_tensor(ea2n[:, c], ea2[:, c], fin_invt[:, c],
                                    op=Alu.mult)
            nc.gpsimd.tensor_tensor(terr[:, c], posn[:, c], xblk[:, c],
                                    op=Alu.subtract)
            nc.scalar.activation(t2[:, c], terr[:, c], Act.Square)
            nc.scalar.activation(p2[:, c], posn[:, c], Act.Square)
            nc.gpsimd.tensor_tensor(vv[:, c], ea2n[:, c], p2[:, c],
                                    op=Alu.subtract)
            nc.vector.scalar_tensor_tensor(r[:, c], vv[:, c], LAM, t2[:, c],
                                           op0=Alu.mult, op1=Alu.add)
            if sl == 1:
                nc.sync.dma_start(d_out[:, 0:2*QS], r[:, 0:2*QS])
            elif sl == 3:
                nc.scalar.dma_start(d_out[:, 2*QS:NQ], r[:, 2*QS:NQ])

        pending = None      # (sq tiles, inv tile) of the in-flight prep window
        inv_t = None
        fin_invt = None
        for k in range(ksteps):
            last = (k == ksteps - 1)
            is_norm = k in norm_set
            prep_next = (k + 1) in norm_set or k == ksteps - 2
            consume = pending is not None
            if consume:
                sqs_c, prs_c, trp_c, inv_t = pending
            wbn = spool.tile([128, 32*NQ], f16, tag="wb")
            if prep_next:
                inv_next = qpool.tile([128, NQ], f32, tag="scl")
                trp_next = smpool.tile([128, NQ], f32, tag="trp")
                sqs_next = []
                prs_next = []
                for i in range(NSLAB):
                    pr_i = qpool.tile([128, QS], f32, tag=f"pr{i}")
                    prs_next.append(pr_i)
            for sl in range(NSLAB):
                pm = pmpool.tile([128, 16*QS], f32, tag="pm")
                for qq in range(QS):
                    q = sl * QS + qq
                    for s in range(4):
                        nc.tensor.matmul(
                            pm[32*s:32*s+32, 16*qq:16*qq+16],
                            wb[32*s:32*s+32, 32*q:32*q+32],
                            wb[32*s:32*s+32, 32*q:32*q+16],
                            start=True, stop=True,
                            tile_position=(32*s, 32*s))
                if consume:
                    if sl < NSLAB - 1:
                        # red[sl+1] one slab ahead of its trp/recip
                        emit_prep_red(sqs_c[sl + 1], prs_c[sl + 1])
                    emit_prep_trp(prs_c[sl], sl, trp_c, inv_t)
                if is_norm:
                    emit_trio(wbn, pm, sl,
                              scl=inv_t[:, sl*QS:(sl+1)*QS],
                              cast_engine="dve_u_act", last=last)
                else:
                    emit_trio(wbn, pm, sl, cast_engine="act", last=last)
                if last:
                    fin_invt = inv_t
                    emit_finish_rowsum(wbn, sl)
                    if sl >= 2:
                        emit_finish_slab(sl - 2)
                if prep_next:
                    sq = qpool.tile([128, 16*QS], f32, tag=f"sq{sl}")
                    emit_prep_sq(pm, sq, wbn, sl)
                    sqs_next.append(sq)
            if prep_next:
                # red[0] of the new window at the prep tail
                emit_prep_red(sqs_next[0], prs_next[0])
                pending = (sqs_next, prs_next, trp_next, inv_next)
            else:
                pending = None
            wb = wbn
            if debug and k == 0:
                nc.sync.dma_start(d_dbg_wb1[:, :], wb[:, :])
            if debug and k == 1:
                nc.sync.dma_start(d_dbg_wb2[:, :], wb[:, :])
            if debug and k == 2:
                nc.sync.dma_start(d_dbg_wb3[:, :], wb[:, :])
            if debug and k >= 3:
                nc.sync.dma_start(d_dbg_wbs[k][:, :], wb[:, :])
            if debug and k == 1:
                sclcp = wpool.tile([128, NQ], f32, tag="sclcp")
                nc.vector.tensor_copy(sclcp[:, :], inv_next[:, :])
                nc.sync.dma_start(d_dbg_scl[:, :], sclcp[:, :])

        # ---------------- finish (emitted interleaved with step 11) --------
        invt = fin_invt
        for sl in range(NSLAB):
            emit_finish_slab(sl)

        if debug:
            nc.sync.dma_start(d_dbg_rs[:, :], rs[:, :])
            nc.sync.dma_start(d_dbg_prt[:, :], invt[:, :])
    nc.compile()
    return nc


def kernel(A_real, A_imag, X):
    from concourse.bass_utils import run_bass_kernel_spmd

    per_core = _build_host_tensors(
        np.asarray(A_real, np.float32), np.asarray(A_imag, np.float32),
        np.asarray(X, np.float32))

    if "nc" not in _prog_cache:
        _prog_cache["nc"] = build_program()
    nc = _prog_cache["nc"]

    res = run_bass_kernel_spmd(nc, per_core, list(range(NCORES)))
    total = 0.0
    for c in range(NCORES):
        total += float(np.asarray(res.results[c]["out"], np.float64).sum())
    return np.float32(total / N)
